# revision 24
# baseline (speedup 1.0000x reference)
"""Trainium2 Bass kernel for nn_NEURAL_PYSCF_WF (neural wavefunction).

reference:
  mo   = einsum('ben,mn->bem', ao, mo_weight)          # [B, 32, 128]
  sub  = mo[:, cfg[:,:,None], cfg[:,None,:]]           # [B, 128, 16, 16]
  dets = det(sub)                                      # [B, 128]
  out  = dets @ ci_weight.T                            # [B, 1]

Config indices are < 32, so only mo[:, :, :32] matters.

Strategy (8 NeuronCores, data-parallel over B=8192). Per core (1024 rows):
  phase 1 (all b-tiles up front): ao (host-cast fp16) -> DMA-XBAR
    transpose -> fp16 matmul (W32T stationary) -> M^T in PSUM ->
    ACT cast fp16 -> DRAM mscr in [b, m, e] layout.
  phase 2 per 128-row b-tile:
    msb [128b, (m,e)] fp16; two-stage GPSIMD gather per 4-config chunk
    (g1 d=32 row gather -> ACT transpose+cast fp32 -> g2 d=16) into
    S [128, 128*256] fp32; reverse-order pivot-free LU (pivot from
    bottom-right so active windows stay 4B-aligned at offset 0),
    batched over 64-config halves; the outer-product (P) and subtract
    passes run in 16-config chunks split across DVE and GPSIMD;
    reciprocal clamped to +-1e6; det = prod(diag) via product tree on
    GPSIMD; out[b] = sum_c ci[c]*det (DVE).
  Gathers for b-tile t+1 are emission-interleaved into t's LU so the
  in-order GPSIMD/ACT queues never stall on the gather chain.
"""

from contextlib import ExitStack

import numpy as np

import concourse.bass as bass
import concourse.bacc as bacc
import concourse.mybir as mybir
import concourse.tile as tile
from concourse.bass_utils import run_bass_kernel_spmd

F32 = mybir.dt.float32
F16 = mybir.dt.float16
I16 = mybir.dt.int16
AX = mybir.AxisListType
OP = mybir.AluOpType

B = 8192
NE = 32      # electrons (and the max config index)
NAO = 128
K = 16       # config size
NCONF = 128
NCORES = 8
BC = B // NCORES
RCLAMP = 1e6

CT = 4                   # configs per gather chunk
NCH = NCONF // CT        # gather chunks per b-tile
CH = 64                  # configs per det product-tree half
# LU chunk c-ranges: each chunk is an independent 15-step pipeline pinned
# to one engine. GPSIMD runs one step skewed behind DVE: its reciprocal
# is emitted at the end of DVE's step block so DVE never head-blocks on
# a GPSIMD dependency. Below TAIL_MM the windows are so small that
# GPSIMD's ~1us/instruction floor loses to DVE: DVE takes over the GPS
# ranges for the remaining steps.
DVE_R = ((0, 16), (16, 32), (32, 48), (48, 64), (64, 80), (80, 96),
         (96, 112), (112, 128))
ND = 128                 # configs on DVE chunks (all: GPSIMD TT stalls
                         # DVE via shared SBUF ports; it keeps gathers)


def wrap_idx(idx: np.ndarray) -> np.ndarray:
    """Wrap a flat index list into ap_gather's [128, n/16] layout."""
    n = idx.shape[0]
    assert n % 16 == 0
    w = idx.reshape(n // 16, 16).T.astype(np.int16)
    return np.tile(w, (8, 1))


def build_gidx1(cfg: np.ndarray) -> np.ndarray:
    """Stage-1 (d=32 rows of Mt): idx[(c,j)] = cfg[c, j]."""
    cols = []
    for cc in range(NCH):
        sl = cfg[cc * CT:(cc + 1) * CT]               # [CT, 16]
        cols.append(wrap_idx(sl.reshape(-1)))         # [128, CT]
    return np.concatenate(cols, axis=1)               # [128, NCONF]


def build_gidx2(cfg: np.ndarray) -> np.ndarray:
    """Stage-2 (d=16 i-runs of Rt): idx[(c,i)] = c*NE + cfg[c, i]."""
    cols = []
    for cc in range(NCH):
        sl = cfg[cc * CT:(cc + 1) * CT]               # [CT, 16]
        idx = (np.arange(CT)[:, None] * NE + sl).reshape(-1)
        cols.append(wrap_idx(idx))                    # [128, CT]
    return np.concatenate(cols, axis=1)               # [128, NCONF]


def emit_program(nc, tc, aps, BCc: int):
    ctx = ExitStack()
    NBT = BCc // 128
    ao, w32t, cirep, gidx1, gidx2, mscr, out = (
        aps["ao"], aps["w32t"], aps["cirep"], aps["gidx1"], aps["gidx2"],
        aps["mscr"], aps["out"])

    with ctx:
        cpool = ctx.enter_context(tc.tile_pool(name="consts", bufs=1))
        aotp = ctx.enter_context(tc.tile_pool(name="aot", bufs=4))
        mtps = ctx.enter_context(
            tc.tile_pool(name="mtps", bufs=2, space="PSUM"))
        mstp = ctx.enter_context(tc.tile_pool(name="mst", bufs=2))
        msbp = ctx.enter_context(tc.tile_pool(name="msb", bufs=2))
        sp = ctx.enter_context(tc.tile_pool(name="S", bufs=1))
        rp = ctx.enter_context(tc.tile_pool(name="r", bufs=2))
        rtp = ctx.enter_context(tc.tile_pool(name="rt", bufs=3))
        up = ctx.enter_context(tc.tile_pool(name="U", bufs=1))
        recp = ctx.enter_context(tc.tile_pool(name="rec", bufs=1))
        pdve = ctx.enter_context(tc.tile_pool(name="pdve", bufs=1))
        pgps = ctx.enter_context(tc.tile_pool(name="pgps", bufs=1))
        detp = ctx.enter_context(tc.tile_pool(name="dets", bufs=2))
        outp = ctx.enter_context(tc.tile_pool(name="outp", bufs=1))

        w32t_s = cpool.tile([128, NE], F16)
        cirep_s = cpool.tile([128, NCONF], F32)
        gidx1_s = cpool.tile([128, NCONF], I16)
        gidx2_s = cpool.tile([128, NCONF], I16)
        clneg = cpool.tile([128, 1], F32)
        clpos = cpool.tile([128, 1], F32)
        clzero = cpool.tile([128, 1], F32)
        nc.sync.dma_start(w32t_s[:], w32t[:])
        nc.sync.dma_start(cirep_s[:], cirep[:])
        nc.sync.dma_start(gidx1_s[:], gidx1[:])
        nc.sync.dma_start(gidx2_s[:], gidx2[:])
        nc.vector.memset(clneg[:], -RCLAMP)
        nc.vector.memset(clpos[:], RCLAMP)
        nc.vector.memset(clzero[:], 0.0)

        out_sb = outp.tile([128, NBT], F32)
        ao3 = ao.rearrange("(t p) n -> t p n", p=128)

        # ------- phase 1: M^T, emitted per b-tile (2 tiles ahead) ---------
        def emit_phase1(bt):
            for g in range(8):          # groups of 16 b (4 ao tiles)
                ps = mtps.tile([NE, 512], F32, name="ps")
                for u in range(4):
                    t = g * 4 + u
                    aot_t = aotp.tile([128, 128], F16, name="aot_t")
                    nc.sync.dma_start_transpose(aot_t[:], ao3[bt * 32 + t])
                    nc.tensor.matmul(
                        ps[:, u * 128:(u + 1) * 128], w32t_s[:], aot_t[:],
                        start=True, stop=True)
                mst = mstp.tile([NE, 512], F16, name="mst")
                nc.scalar.copy(mst[:], ps[:])
                # dst iterates (m, b16, e32) to match the [32p, 512] src
                dst = bass.AP(
                    mscr.tensor, (bt * 128 + g * 16) * (NE * NE),
                    [[NE, NE], [NE * NE, 16], [1, NE]])
                nc.scalar.dma_start(dst, mst[:])

        emit_phase1(0)
        emit_phase1(1)

        # ---------------- phase 2 ----------------------------------------
        def emit_g1T(bt, cc, r_tiles, on_dve=False):
            """Stage-1 gather + transpose for chunk cc of b-tile bt."""
            r_t = rp.tile([128, CT * K * NE], F16)
            nc.gpsimd.ap_gather(
                r_t[:], r_tiles[bt][:],
                gidx1_s[:, cc * CT:(cc + 1) * CT],
                channels=128, num_elems=NE, d=NE, num_idxs=CT * K)
            rt_t = rtp.tile([128, CT * K * NE], F32)
            rt_dst = bass.AP(
                rt_t[:].tensor, rt_t[:].offset,
                [[int(rt_t[:].ap[0][0]), 128],
                 [K * NE, CT], [K, NE], [1, K]])
            r_src = bass.AP(
                r_t[:].tensor, r_t[:].offset,
                [[int(r_t[:].ap[0][0]), 128],
                 [K * NE, CT], [1, NE], [NE, K]])
            if on_dve:
                nc.vector.tensor_tensor(
                    rt_dst, r_src,
                    clzero[:].broadcast_to([128, CT * K * NE]), op=OP.add)
            else:
                nc.scalar.copy(rt_dst, r_src)
            return rt_t

        def emit_g2(bt, cc, rt_t, s_t):
            nc.gpsimd.ap_gather(
                s_t[:, cc * CT * K * K:(cc + 1) * CT * K * K], rt_t[:],
                gidx2_s[:, cc * CT:(cc + 1) * CT],
                channels=128, num_elems=CT * NE, d=K, num_idxs=CT * K)

        msb_tiles = {}
        s_tiles = {}
        rt_store = {}

        def load_msb(bt):
            m = msbp.tile([128, NE * NE], F16)
            src = bass.AP(
                mscr.tensor, bt * 128 * (NE * NE),
                [[NE * NE, 128], [1, NE * NE]])
            nc.scalar.dma_start(m[:], src)
            msb_tiles[bt] = m

        def gather_iter(bt, alt=False):
            """Yields once per g1+T chunk emitted for b-tile bt."""
            rts = []
            for cc in range(NCH):
                rts.append(emit_g1T(bt, cc, msb_tiles,
                                    on_dve=alt and cc % 2 == 1))
                yield
            rt_store[bt] = rts

        def emit_g2_all(bt):
            s_t = sp.tile([128, NCONF * K * K], F32)
            s_tiles[bt] = s_t
            for cc in range(NCH):
                emit_g2(bt, cc, rt_store[bt][cc], s_t)
            del rt_store[bt]

        def emit_lu(bt, bg_iter):
            """15-step reverse LU on s_tiles[bt]; sprinkles gather chunks
            for the next b-tile (bg_iter) between steps."""
            s_t = s_tiles[bt]
            S4 = s_t[:].rearrange("p (c i j) -> p c i j", i=K, j=K)
            rd_raw = recp.tile([128, ND], F32, tag="rd_raw", name="rdr")
            rd = recp.tile([128, ND], F32, tag="rd", name="rd")
            ud_t = up.tile([128, ND * (K - 1)], F32, tag="ud", name="ud")
            U3d = ud_t[:].rearrange("p (c i) -> p c i", c=ND)

            def clamp(dst, src, n):
                # min/max via TENSOR_TENSOR: tensor_scalar would engage
                # the DVE 2-port perf mode, which fully blocks GPSIMD's
                # shared SBUF port pair while it runs.
                nc.vector.tensor_tensor(
                    dst, src,
                    clneg[:].broadcast_to([128, n]), op=OP.max)
                nc.vector.tensor_tensor(
                    dst, dst,
                    clpos[:].broadcast_to([128, n]), op=OP.min)

            def emit_pair(eng, pool, c0, c1, Usrc):
                cn = c1 - c0
                cmax = 16
                p_t = pool.tile([128, cmax * (K - 1) * (K - 1)], F32,
                                name="ptile")
                P4 = p_t[:].rearrange(
                    "p (c i j) -> p c i j", i=K - 1, j=K - 1)[:, :cn]
                col = S4[:, c0:c1, :mm, mm]
                eng.tensor_tensor(
                    P4[:, :, :mm, :mm],
                    col.unsqueeze(3).broadcast_to([128, cn, mm, mm]),
                    Usrc.unsqueeze(2).broadcast_to([128, cn, mm, mm]),
                    op=OP.mult)
                Sw = S4[:, c0:c1, :mm, :mm]
                eng.tensor_tensor(
                    Sw, Sw, P4[:, :, :mm, :mm], op=OP.subtract)

            for step in range(K - 1):
                mm = K - 1 - step
                if step <= 2:
                    # Per-chunk rec/clamp/U so DVE chunk c starts as soon
                    # as ITS g2 gathers land (no all-chunk barrier at the
                    # b-tile boundary).
                    for c0, c1 in DVE_R:
                        nc.vector.reciprocal(
                            rd_raw[:, c0:c1], S4[:, c0:c1, mm, mm])
                        clamp(rd[:, c0:c1], rd_raw[:, c0:c1], c1 - c0)
                        nc.vector.tensor_tensor(
                            U3d[:, c0:c1, :mm], S4[:, c0:c1, mm, :mm],
                            rd[:, c0:c1].unsqueeze(2).broadcast_to(
                                [128, c1 - c0, mm]),
                            op=OP.mult)
                        emit_pair(nc.vector, pdve, c0, c1,
                                  U3d[:, c0:c1, :mm])
                else:
                    # All deps DVE-internal: batched, no stalls.
                    nc.vector.reciprocal(rd_raw[:], S4[:, 0:ND, mm, mm])
                    clamp(rd[:], rd_raw[:], ND)
                    nc.vector.tensor_tensor(
                        U3d[:, :, :mm], S4[:, 0:ND, mm, :mm],
                        rd[:].unsqueeze(2).broadcast_to([128, ND, mm]),
                        op=OP.mult)
                    for c0, c1 in DVE_R:
                        emit_pair(nc.vector, pdve, c0, c1,
                                  U3d[:, c0:c1, :mm])
                # background gather chunks for the next b-tile
                if bg_iter is not None:
                    for _ in range(3):
                        if next(bg_iter, "done") == "done":
                            bg_iter = None
                            break

            while bg_iter is not None and next(bg_iter, "done") != "done":
                pass

            # det = prod(diag) via product tree (GPSIMD)
            dets_t = detp.tile([128, NCONF], F32)
            base = s_t[:]
            pstride = int(base.ap[0][0])
            for h in range(2):
                off = base.offset + h * CH * K * K
                t8 = pgps.tile([128, CH * 8], F32, tag=f"t8{h}")
                nc.vector.tensor_tensor(
                    t8[:].rearrange("p (c x) -> p c x", c=CH),
                    bass.AP(base.tensor, off,
                            [[pstride, 128], [K * K, CH], [2 * (K + 1), 8]]),
                    bass.AP(base.tensor, off + (K + 1),
                            [[pstride, 128], [K * K, CH], [2 * (K + 1), 8]]),
                    op=OP.mult)
                t4 = pgps.tile([128, CH * 4], F32, tag=f"t4{h}")
                nc.vector.tensor_tensor(
                    t4[:].rearrange("p (c x) -> p c x", c=CH),
                    bass.AP(t8[:].tensor, t8[:].offset,
                            [[int(t8[:].ap[0][0]), 128], [8, CH], [2, 4]]),
                    bass.AP(t8[:].tensor, t8[:].offset + 1,
                            [[int(t8[:].ap[0][0]), 128], [8, CH], [2, 4]]),
                    op=OP.mult)
                t2 = pgps.tile([128, CH * 2], F32, tag=f"t2{h}")
                nc.vector.tensor_tensor(
                    t2[:].rearrange("p (c x) -> p c x", c=CH),
                    bass.AP(t4[:].tensor, t4[:].offset,
                            [[int(t4[:].ap[0][0]), 128], [4, CH], [2, 2]]),
                    bass.AP(t4[:].tensor, t4[:].offset + 1,
                            [[int(t4[:].ap[0][0]), 128], [4, CH], [2, 2]]),
                    op=OP.mult)
                nc.vector.tensor_tensor(
                    dets_t[:, h * CH:(h + 1) * CH],
                    bass.AP(t2[:].tensor, t2[:].offset,
                            [[int(t2[:].ap[0][0]), 128], [2, CH]]),
                    bass.AP(t2[:].tensor, t2[:].offset + 1,
                            [[int(t2[:].ap[0][0]), 128], [2, CH]]),
                    op=OP.mult)

            wd = detp.tile([128, NCONF], F32, tag="wd")
            nc.vector.tensor_tensor(
                wd[:], dets_t[:], cirep_s[:], op=OP.mult)
            nc.vector.tensor_reduce(
                out_sb[:, bt:bt + 1], wd[:], axis=AX.X, op=OP.add)
            del s_tiles[bt]

        # startup: gathers for bt 0
        load_msb(0)
        it0 = gather_iter(0, alt=True)
        while next(it0, "done") != "done":
            pass
        for bt in range(NBT):
            emit_g2_all(bt)
            if bt + 2 < NBT:
                emit_phase1(bt + 2)
            if bt + 1 < NBT:
                load_msb(bt + 1)
                bg = gather_iter(bt + 1)
            else:
                bg = None
            emit_lu(bt, bg)

        nc.sync.dma_start(out[:], out_sb[:])


def build(BCc: int):
    nc = bacc.Bacc("TRN2", target_bir_lowering=False, debug=False)
    aps = {}
    aps["ao"] = nc.dram_tensor(
        "ao", [BCc * NE, NAO], F16, kind="ExternalInput").ap()
    aps["w32t"] = nc.dram_tensor(
        "w32t", [NAO, NE], F16, kind="ExternalInput").ap()
    aps["cirep"] = nc.dram_tensor(
        "cirep", [128, NCONF], F32, kind="ExternalInput").ap()
    aps["gidx1"] = nc.dram_tensor(
        "gidx1", [128, NCONF], I16, kind="ExternalInput").ap()
    aps["gidx2"] = nc.dram_tensor(
        "gidx2", [128, NCONF], I16, kind="ExternalInput").ap()
    aps["mscr"] = nc.dram_tensor("mscr", [BCc * NE * NE], F16).ap()
    aps["out"] = nc.dram_tensor(
        "out", [128, BCc // 128], F32, kind="ExternalOutput").ap()

    with tile.TileContext(nc) as tc:
        emit_program(nc, tc, aps, BCc)
    nc.compile()
    return nc


def host_inputs(ao_shard, mo_weight, ci_weight, configs):
    BCc = ao_shard.shape[0]
    w32 = mo_weight[:NE, :]
    return {
        "ao": np.ascontiguousarray(
            ao_shard.reshape(BCc * NE, NAO)).astype(np.float16),
        "w32t": np.ascontiguousarray(w32.T).astype(np.float16),
        "cirep": np.ascontiguousarray(
            np.tile(ci_weight.astype(np.float32), (128, 1))),
        "gidx1": build_gidx1(configs),
        "gidx2": build_gidx2(configs),
    }


_CACHE: dict = {}


def _get_program():
    key = ("prog", BC, CT, ND)
    if key not in _CACHE:
        _CACHE[key] = build(BC)
    return _CACHE[key]


def kernel(ao, mo_weight, ci_weight, configs):
    ao = np.asarray(ao, dtype=np.float32)
    mo_weight = np.asarray(mo_weight, dtype=np.float32)
    ci_weight = np.asarray(ci_weight, dtype=np.float32)
    configs = np.asarray(configs, dtype=np.int32)
    assert ao.shape == (B, NE, NAO)

    nc = _get_program()
    in_maps = [
        host_inputs(ao[c * BC:(c + 1) * BC], mo_weight, ci_weight, configs)
        for c in range(NCORES)
    ]
    res = run_bass_kernel_spmd(nc, in_maps, core_ids=list(range(NCORES)))
    outs = []
    for c in range(NCORES):
        o = np.asarray(res.results[c]["out"])      # [128, NBT]
        outs.append(o.T.reshape(-1))               # b = bt*128 + p
    return np.concatenate(outs).astype(np.float32)[:, None]


def ref_algo(ao_shard, mo_weight, ci_weight, configs):
    """Numpy replica of the on-device algorithm (dev checking only)."""
    ao16 = ao_shard.astype(np.float16).astype(np.float32)
    w16 = mo_weight[:NE].astype(np.float16).astype(np.float32)
    M = np.einsum("ben,mn->bem", ao16, w16).astype(np.float32)
    M = M.astype(np.float16).astype(np.float32)
    S = M[:, configs[:, :, None], configs[:, None, :]].astype(np.float32)
    Bs = S.shape[0]
    A = S.reshape(-1, K, K).copy()
    rcl = np.float32(RCLAMP)
    for step in range(K - 1):
        mm = K - 1 - step
        piv = A[:, mm, mm].copy()
        with np.errstate(divide="ignore"):
            rec = (np.float32(1.0) / piv).astype(np.float32)
        rec = np.clip(rec, -rcl, rcl)
        L = (A[:, :mm, mm] * rec[:, None]).astype(np.float32)
        A[:, :mm, :mm] -= (
            L[:, :, None] * A[:, mm, None, :mm]).astype(np.float32)
    diag = A[:, np.arange(K), np.arange(K)]
    t8 = diag[:, 0::2] * diag[:, 1::2]
    t4 = t8[:, 0::2] * t8[:, 1::2]
    t2 = t4[:, 0::2] * t4[:, 1::2]
    det = (t2[:, 0] * t2[:, 1]).astype(np.float32)
    dets_ = det.reshape(Bs, NCONF)
    return (dets_ @ ci_weight.T.astype(np.float32)).astype(np.float32)
